# revision 46
# baseline (speedup 1.0000x reference)
"""Trainium2 Bass kernel for the dense-MoE problem (nn_MoE_20899310862533).

Contract: kernel(**inputs) takes the FULL unsharded inputs (keys as in
reference.setup_inputs()) and returns the FULL [32768, 256] float32 output.

Strategy: data-parallel over the batch across 8 NeuronCores (weights
replicated, no collectives). On chip everything is feature-major
([feature, batch]); the host passes x pre-transposed per shard and
transposes the per-core output back.

Design (HW-measured: fp32r matmuls stream ~1 col/cycle at N>=256; LDWEIGHTS
is exposed (~11 ns/MM) only when the stationary rotates EVERY matmul, and
pairing two moving tiles per stationary removes it; sustained execution
throttles to a flat ~0.55 ns/col column-rate wall regardless of dtype):
  * Batch tiles processed in PAIRS (A, B): each expert-weight block loads
    once and streams two 512-wide moving tiles back to back.
  * Uniform cross-pair software pipeline: per (e, ht) emit 4 l1 matmuls
    (kd x A/B) -> ACT relu into a1 -> DVE in-place gate-mul -> 4 l2
    matmuls for the PREVIOUS (e, ht) (possibly of the previous pair, so
    there is no drain bubble at pair boundaries). Output PSUM banks are
    single-bank tiles (pout bufs=5 / pbig bufs=3) so a new pair's first
    l2 never waits on the old pair's epilogue.
  * Softmax-free gating: logits computed directly in [E=16 partitions,
    512 batch] layout (stationary = Wg4, one N=512 matmul instead of 4
    batch-major N=16 matmuls + 4 PE transposes), LeakyReLU as ACT
    Identity+bias then DVE max (beats ACT Lrelu: fewer ACT table
    switches), exp WITHOUT max-subtraction (logits are
    O(5), no overflow risk), partition-dim sum via a ones-matmul, and
    the 1/sum normalization applied to the OUTPUT tiles via a
    DRAM-roundtrip broadcast of the reciprocal row. Gating for the next
    pair is interleaved into the expert loop as 10 stages.
  * Gate rows and reciprocal rows broadcast across 128 partitions with
    DRAM round-trip DMAs (gscr/rscr), prefetched 2 experts / 1 pair
    ahead; pair-0 state (x tiles, gscr/rscr) is produced by the prologue
    and persists across For_i reps.
  * All timing-sensitive engine choices are HW-validated: GPSIMD cannot
    read PSUM (epilogue muls on DVE), gpsimd tensor ops are ~10x slower
    than DVE (gate-muls on DVE), bf16 gates make mixed-dtype DVE muls
    slow (everything f32r).
"""

import numpy as np

import concourse.bass as bass
import concourse.mybir as mybir
import concourse.tile as tile
from concourse import bacc
from concourse.bass_utils import run_bass_kernel_spmd

F32 = mybir.dt.float32
F32R = mybir.dt.float32r
AF = mybir.ActivationFunctionType
ALU = mybir.AluOpType
AX = mybir.AxisListType

B, D, H, O, E = 32768, 256, 512, 256, 16
NCORES = 8
BC = B // NCORES      # 4096 batch rows per core
BT = 512              # batch tile (moving/free dim)
NBT = BC // BT        # 8 tiles -> 4 pairs
NPAIR = NBT // 2
KD = D // 128         # 2 contraction tiles for D
HT = H // 128         # 4 tiles for H
OT = O // 128         # 2 tiles for O

MMDT = F32R
BF16 = mybir.dt.bfloat16
GBDT = F32R           # gate scratch/broadcast dtype (bf16 was slower: mixed-dtype DVE muls)
USE_ACT_LRELU = False  # Identity+DVE max beats ACT Lrelu on HW (ACT table switches)


def build_program(n_reps=1, has_be1=False, has_be2=False, has_bg4=False,
                  psum_cfg=None, unroll=False, l2_split=1, has_bg123=False):
    nc = bacc.Bacc("TRN2", target_bir_lowering=False, debug=False,
                   num_devices=NCORES)

    def din(name, shape, dt=F32):
        return nc.dram_tensor(name, shape, dt, kind="ExternalInput").ap()

    xT = din("xT", [D, BC], MMDT)
    We1 = din("We1", [E, D, H], MMDT)
    We2 = din("We2", [E, H, O], MMDT)
    be1T = din("be1T", [H, E])
    be2 = din("be2", [E, O], GBDT)
    Wg1 = din("Wg1", [D, 128], MMDT)
    Wg2 = din("Wg2", [128, 256], MMDT)
    Wg3 = din("Wg3", [256, 128], MMDT)
    Wg4 = din("Wg4", [128, E], MMDT)
    bg1 = din("bg1", [128, 1])
    bg2 = din("bg2", [256, 1])
    bg3 = din("bg3", [128, 1])
    bg4c = din("bg4c", [E, 1])
    ones16 = din("ones16", [E, 1], GBDT)
    ident = din("ident", [128, 128])
    outT = nc.dram_tensor("outT", [O, BC], F32, kind="ExternalOutput").ap()
    gscr = nc.dram_tensor("gscr", [NBT, E, BT], GBDT, kind="Internal").ap()
    rscr = nc.dram_tensor("rscr", [NBT, 1, BT], F32, kind="Internal").ap()

    with tile.TileContext(nc) as tc:
        from contextlib import ExitStack
        with ExitStack() as ctx:
            const = ctx.enter_context(tc.tile_pool(name="const", bufs=1))
            xpool = ctx.enter_context(tc.tile_pool(name="x", bufs=4))
            gpool = ctx.enter_context(tc.tile_pool(name="gate", bufs=2))
            gtpool = ctx.enter_context(tc.tile_pool(name="gt", bufs=2))
            rspool = ctx.enter_context(tc.tile_pool(name="rs", bufs=1))
            rsbpool = ctx.enter_context(tc.tile_pool(name="rsb", bufs=3))
            gbpool = ctx.enter_context(tc.tile_pool(name="gb", bufs=6))
            apool = ctx.enter_context(tc.tile_pool(name="a1s", bufs=4))
            opool = ctx.enter_context(tc.tile_pool(name="osb", bufs=4))
            pbig = ctx.enter_context(
                tc.tile_pool(name="pbig", bufs=3, space="PSUM"))
            pout = ctx.enter_context(
                tc.tile_pool(name="pout", bufs=5, space="PSUM"))

            def load_x(bt, split=1, eng=None):
                xs = xpool.tile([128, KD, BT], MMDT, name="xs")
                w = BT // split
                for kd in range(KD):
                    for s in range(split):
                        (eng or nc.sync).dma_start(
                            out=xs[:, kd, s * w:(s + 1) * w],
                            in_=xT[kd * 128:(kd + 1) * 128,
                                   bt * BT + s * w:bt * BT + (s + 1) * w])
                return xs

            # ---- gating-critical constants + pair-0 x tiles first so the
            #      PE starts as early as possible ----
            wg1_s = const.tile([128, KD, 128], MMDT, name="wg1_s")
            for kd in range(KD):
                nc.sync.dma_start(out=wg1_s[:, kd, :],
                                  in_=Wg1[kd * 128:(kd + 1) * 128, :])
            bg1_s = const.tile([128, 1], F32, name="bg1_s")
            nc.scalar.dma_start(out=bg1_s, in_=bg1)
            id_s = const.tile([128, 128], F32, name="id_s")
            nc.scalar.dma_start(out=id_s, in_=ident)
            def load_x_const(bt, name):
                xs = const.tile([128, KD, BT], MMDT, name=name)
                w = BT // 2
                for kd in range(KD):
                    for sp in range(2):
                        nc.sync.dma_start(
                            out=xs[:, kd, sp * w:(sp + 1) * w],
                            in_=xT[kd * 128:(kd + 1) * 128,
                                   bt * BT + sp * w:bt * BT + (sp + 1) * w])
                return xs

            xs0a = load_x_const(0, "xs0a")
            xs0b = load_x_const(1, "xs0b")

            # PE p-state warm-up while the first DMAs land
            for _ in range(8):
                pdum = pbig.tile([128, BT], F32, name="pb")
                nc.tensor.matmul(pdum[:, :128], id_s, id_s,
                                 start=True, stop=True)

            wg2_s = const.tile([128, 256], MMDT, name="wg2_s")
            nc.sync.dma_start(out=wg2_s, in_=Wg2)
            wg3_s = const.tile([128, KD, 128], MMDT, name="wg3_s")
            for kd in range(KD):
                nc.sync.dma_start(out=wg3_s[:, kd, :],
                                  in_=Wg3[kd * 128:(kd + 1) * 128, :])
            wg4_s = const.tile([128, E], MMDT, name="wg4_s")
            nc.sync.dma_start(out=wg4_s, in_=Wg4)
            bg2_s = const.tile([128, 2], F32, name="bg2_s")
            for m in range(2):
                nc.sync.dma_start(out=bg2_s[:, m:m + 1],
                                  in_=bg2[m * 128:(m + 1) * 128, :])
            bg3_s = const.tile([128, 1], F32, name="bg3_s")
            nc.sync.dma_start(out=bg3_s, in_=bg3)
            ones16_s = const.tile([E, 1], GBDT, name="ones16_s")
            nc.sync.dma_start(out=ones16_s, in_=ones16)
            bg4c_s = None
            if has_bg4:
                bg4c_s = const.tile([E, 1], F32, name="bg4c_s")
                nc.sync.dma_start(out=bg4c_s, in_=bg4c)
            be1t_s = None
            if has_be1:
                be1t_s = const.tile([128, HT, E], F32, name="be1t_s")
                for ht in range(HT):
                    nc.sync.dma_start(out=be1t_s[:, ht, :],
                                      in_=be1T[ht * 128:(ht + 1) * 128, :])
            be2_s = None
            if has_be2:
                be2_s = const.tile([E, O], GBDT, name="be2_s")
                nc.sync.dma_start(out=be2_s, in_=be2)

            def lrelu_from_psum(dst, psum, bias_col):
                if USE_ACT_LRELU:
                    nc.scalar.activation(out=dst, in_=psum, func=AF.Lrelu,
                                         bias=bias_col, scale=1.0, alpha=0.01)
                else:
                    nc.scalar.activation(out=dst, in_=psum, func=AF.Identity,
                                         bias=bias_col, scale=1.0)
                    nc.vector.scalar_tensor_tensor(
                        out=dst, in0=dst, scalar=0.01, in1=dst,
                        op0=ALU.mult, op1=ALU.max)

            def gating_stages(bt, xs, holder, prologue=False):
                """Gating for one batch tile, split into 5 emission stages.
                Produces gscr[bt] (unnormalized exp-gates, [E, BT]) and a
                broadcast [128, BT] reciprocal-sum tile in holder['rsb']."""
                wdma = nc.scalar if prologue else nc.sync

                def s1():
                    pg = pbig.tile([128, BT], F32, name="pb")
                    for kd in range(KD):
                        nc.tensor.matmul(pg, wg1_s[:, kd, :], xs[:, kd, :],
                                         start=(kd == 0), stop=(kd == KD - 1))
                    g1 = gpool.tile([128, BT], MMDT, name="g1")
                    lrelu_from_psum(g1, pg, bg1_s)
                    holder["g1"] = g1

                def s2():
                    g1 = holder["g1"]
                    g2 = gpool.tile([128, 2, BT], MMDT, name="g2")
                    for m in range(2):
                        pg2 = pbig.tile([128, BT], F32, name="pb")
                        nc.tensor.matmul(pg2,
                                         wg2_s[:, m * 128:(m + 1) * 128],
                                         g1, start=True, stop=True)
                        lrelu_from_psum(g2[:, m, :], pg2, bg2_s[:, m:m + 1])
                    holder["g2"] = g2

                def s3():
                    g2 = holder["g2"]
                    pg3 = pbig.tile([128, BT], F32, name="pb")
                    for kd in range(2):
                        nc.tensor.matmul(pg3, wg3_s[:, kd, :], g2[:, kd, :],
                                         start=(kd == 0), stop=(kd == 1))
                    g3 = gpool.tile([128, BT], MMDT, name="g3")
                    lrelu_from_psum(g3, pg3, bg3_s)
                    holder["g3"] = g3

                def s4():
                    # logits directly in [E, BT] layout; exp without
                    # max-subtraction (|logit| is O(5); exp fits fp32 easily)
                    g3 = holder["g3"]
                    plog = pbig.tile([128, BT], F32, name="pb")
                    nc.tensor.matmul(plog[:E, :], wg4_s, g3,
                                     start=True, stop=True)
                    GT = gtpool.tile([E, BT], GBDT, name="GT")
                    nc.scalar.activation(
                        out=GT, in_=plog[:E, :], func=AF.Exp,
                        bias=(bg4c_s if has_bg4 else 0.0), scale=1.0)
                    holder["GT"] = GT
                    wdma.dma_start(out=gscr[bt], in_=GT)

                def s5():
                    GT = holder["GT"]
                    ps = pbig.tile([128, BT], F32, name="pb")
                    nc.tensor.matmul(ps[:1, :], ones16_s, GT,
                                     start=True, stop=True)
                    rs = rspool.tile([1, BT], F32, name="rs")
                    nc.vector.reciprocal(rs, ps[:1, :])
                    wdma.dma_start(out=rscr[bt], in_=rs)

                return [s1, s2, s3, s4, s5]

            # ---- prologue: gating for pair 0 (tiles 0 and 1), two chains
            #      interleaved so PE/ACT pipeline ----
            hold0a, hold0b = {}, {}
            st_a = gating_stages(0, xs0a, hold0a, prologue=True)
            st_b = gating_stages(1, xs0b, hold0b, prologue=True)
            for sa, sb in zip(st_a, st_b):
                sa()
                sb()

            # ---- expert weights: front-load we1, interleave we2 across the
            #      SP-HWDGE and SWDGE rings; gating/gb round-trips ride the
            #      Activation HWDGE rings so they never queue behind these ----
            wdma_ct = 0

            def wload(tile_, src, parts):
                nonlocal wdma_ct
                for p in range(parts):
                    eng = nc.sync if wdma_ct % 2 == 0 else nc.gpsimd
                    eng.dma_start(out=tile_[:, p, :], in_=src[p])
                    wdma_ct += 1

            we1_s = [const.tile([128, KD, H], MMDT, name=f"we1_{e}")
                     for e in range(E)]
            we2_s = [const.tile([128, HT, O], MMDT, name=f"we2_{e}")
                     for e in range(E)]

            def w1src(e):
                return [We1[e, kd * 128:(kd + 1) * 128, :] for kd in range(KD)]

            def w2src(e):
                return [We2[e, ht * 128:(ht + 1) * 128, :] for ht in range(HT)]

            # strict consumption order: v2's pipeline needs we2[e] one
            # ht-slot after we1[e] (l2 trails l1 by a single slot), so
            # interleave per expert instead of front-loading all we1.
            # we1[e+1] is prefetched before we2[e]'s tail to keep l1 fed.
            defer_from = 8 if n_reps == 1 else E
            wload(we1_s[0], w1src(0), KD)
            for e in range(E):
                if e + 1 < E:
                    wload(we1_s[e + 1], w1src(e + 1), KD)
                if e < defer_from:
                    wload(we2_s[e], w2src(e), HT)

            def wload_tail():
                for e in range(defer_from, E):
                    wload(we2_s[e], w2src(e), HT)

            def run_body():
                xs_nxt = [xs0a, xs0b]
                rsbs = {}
                gbs = {}

                def rsb_issue(p, scalar_ring=False):
                    # broadcast 1/sumexp rows for both tiles of pair p from
                    # the persistent DRAM scratch (written by stage s5)
                    for t, nm in ((2 * p, "A"), (2 * p + 1, "B")):
                        rsb = rsbpool.tile([128, BT], F32, name="rsb")
                        eng = nc.scalar if scalar_ring else nc.sync
                        eng.dma_start(
                            out=rsb,
                            in_=rscr[t, 0:1, :].partition_broadcast(128))
                        rsbs[(p, nm)] = rsb

                def gb_issue(p, e, scalar_ring=False):
                    # broadcast gate row e for both tiles of pair p
                    if p >= NPAIR or e >= E:
                        return
                    for t, nm in ((2 * p, "A"), (2 * p + 1, "B")):
                        gb = gbpool.tile([128, BT], GBDT, name="gb")
                        eng = nc.scalar if scalar_ring else nc.sync
                        eng.dma_start(
                            out=gb,
                            in_=gscr[t, e:e + 1, :].partition_broadcast(128))
                        gbs[(p, nm, e)] = gb

                for e in range(2):
                    gb_issue(0, e, scalar_ring=True)
                rsb_issue(0, scalar_ring=True)

                def epi_unit(t, po_ot, rsb, ot):
                    # normalize one output bank by broadcast 1/sum (DVE;
                    # GPSIMD cannot read PSUM on hardware), DMA in halves;
                    # half-sized osb tiles x4 bufs pipeline mul->DMA deeper
                    for hb2 in range(2):
                        cs = slice(hb2 * (BT // 2), (hb2 + 1) * (BT // 2))
                        osb = opool.tile([128, BT // 2], F32, name="osb")
                        nc.vector.tensor_mul(osb, po_ot[:, cs], rsb[:, cs])
                        nc.sync.dma_start(
                            out=outT[ot * 128:(ot + 1) * 128,
                                     t * BT + hb2 * (BT // 2):
                                     t * BT + (hb2 + 1) * (BT // 2)],
                            in_=osb)

                def epilogue(p, poA, poB):
                    if has_be2:
                        for t, po in ((2 * p, poA), (2 * p + 1, poB)):
                            GTt = gtpool.tile([E, BT], GBDT, name="GTr")
                            nc.sync.dma_start(out=GTt, in_=gscr[t])
                            for ot in range(OT):
                                nc.tensor.matmul(
                                    po[ot],
                                    be2_s[:, ot * 128:(ot + 1) * 128],
                                    GTt, start=False, stop=True)
                    for t, po, rsb in ((2 * p, poA, rsbs[(p, "A")]),
                                       (2 * p + 1, poB, rsbs[(p, "B")])):
                        for ot in range(OT):
                            epi_unit(t, po[ot], rsb, ot)

                def emit_l2(pd):
                    # l2 matmuls for a pending (pair, e, ht) a1 pair; when it
                    # closes a pair's accumulation, run that pair's epilogue
                    poA_, poB_, aA, aB, pe, pht, pp = pd
                    first = (pe == 0 and pht == 0)
                    last = (pe == E - 1 and pht == HT - 1)
                    for ot in range(OT):
                        w2 = we2_s[pe][:, pht, ot * 128:(ot + 1) * 128]
                        nc.tensor.matmul(poA_[ot], w2, aA,
                                         start=first,
                                         stop=(last and not has_be2))
                        nc.tensor.matmul(poB_[ot], w2, aB,
                                         start=first,
                                         stop=(last and not has_be2))
                        if last and not has_be2:
                            epi_unit(2 * pp, poA_[ot], rsbs[(pp, "A")], ot)
                            epi_unit(2 * pp + 1, poB_[ot], rsbs[(pp, "B")],
                                     ot)
                    if last and has_be2:
                        epilogue(pp, poA_, poB_)

                pend = None  # pending l2: (poA, poB, a1A, a1B, e, ht, p)
                for p in range(NPAIR):
                    tA, tB = 2 * p, 2 * p + 1
                    xsA, xsB = xs_nxt
                    poA = [pout.tile([128, BT], F32, name="po")
                           for _ in range(OT)]
                    poB = [pout.tile([128, BT], F32, name="po")
                           for _ in range(OT)]

                    stages = None
                    if p + 1 < NPAIR:
                        ha, hb = {}, {}
                        stages = []

                    for e in range(E):
                        # prefetch gate broadcasts 3 experts ahead; cross
                        # into the next pair so its first experts never
                        # wait on the DRAM round trip
                        if e + 2 < E:
                            gb_issue(p, e + 2)
                        else:
                            gb_issue(p + 1, e + 2 - E)
                        if p + 1 < NPAIR:
                            if e == 2:
                                xs_n0 = load_x(2 * (p + 1))
                                xs_n1 = load_x(2 * (p + 1) + 1)
                                xs_nxt = [xs_n0, xs_n1]
                                sa = gating_stages(2 * (p + 1), xs_n0, ha)
                                sb = gating_stages(2 * (p + 1) + 1, xs_n1, hb)
                                stages = [f for pr in zip(sa, sb) for f in pr]
                            if stages and 3 <= e < 3 + len(stages):
                                stages[e - 3]()
                        if p == 0 and e == 5:
                            wload_tail()
                        if e == 13 and p + 1 < NPAIR:
                            # after stage s5B (e==12) has written rscr
                            rsb_issue(p + 1)
                        for ht in range(HT):
                            # l1 for (e, ht): 4 MMs, stationary reused A/B
                            p1A = pbig.tile([128, BT], F32, name="pb")
                            p1B = pbig.tile([128, BT], F32, name="pb")
                            for kd in range(KD):
                                w1 = we1_s[e][:, kd, ht * 128:(ht + 1) * 128]
                                nc.tensor.matmul(p1A, w1, xsA[:, kd, :],
                                                 start=(kd == 0),
                                                 stop=(kd == KD - 1))
                                nc.tensor.matmul(p1B, w1, xsB[:, kd, :],
                                                 start=(kd == 0),
                                                 stop=(kd == KD - 1))
                            # relu(psum) straight into the a1 tile (ACT),
                            # then gate-scale in place (DVE tensor-tensor,
                            # the fast 8-lane path)
                            b1 = (be1t_s[:, ht, e:e + 1] if has_be1 else 0.0)
                            a1A = apool.tile([128, BT], MMDT, name="a1")
                            a1B = apool.tile([128, BT], MMDT, name="a1")
                            nc.scalar.activation(out=a1A, in_=p1A,
                                                 func=AF.Relu, bias=b1,
                                                 scale=1.0)
                            nc.scalar.activation(out=a1B, in_=p1B,
                                                 func=AF.Relu, bias=b1,
                                                 scale=1.0)
                            nc.vector.tensor_mul(a1A, a1A, gbs[(p, "A", e)])
                            nc.vector.tensor_mul(a1B, a1B, gbs[(p, "B", e)])
                            # l2 for the previous (e, ht) — possibly of the
                            # PREVIOUS pair: the pipeline is uniform across
                            # pair boundaries, so there is no drain bubble
                            if pend is not None:
                                emit_l2(pend)
                            pend = (poA, poB, a1A, a1B, e, ht, p)

                # rep-boundary drain: close the last pair
                emit_l2(pend)

            if n_reps > 1:
                if unroll:
                    for _ in range(n_reps):
                        run_body()
                else:
                    with tc.For_i(0, n_reps, 1):
                        run_body()
            else:
                run_body()

    nc.compile()
    return nc


_program_cache = {}


def get_program(has_be1=False, has_be2=False, has_bg4=False,
                has_bg123=False):
    key = (has_be1, has_be2, has_bg4, has_bg123)
    if key not in _program_cache:
        _program_cache[key] = build_program(
            has_be1=has_be1, has_be2=has_be2, has_bg4=has_bg4,
            has_bg123=has_bg123)
    return _program_cache[key]


def make_in_maps(inputs):
    f = lambda a: np.ascontiguousarray(np.asarray(a, dtype=np.float32))
    x = f(inputs["x"])
    shared = {
        "We1": f(inputs["We1"]),
        "We2": f(inputs["We2"]),
        "be1T": f(inputs["be1"]).T.copy(),
        "be2": f(inputs["be2"]),
        "Wg1": f(inputs["Wg1"]),
        "Wg2": f(inputs["Wg2"]),
        "Wg3": f(inputs["Wg3"]),
        "Wg4": f(inputs["Wg4"]),
        "bg1": f(inputs["bg1"]).reshape(128, 1),
        "bg2": f(inputs["bg2"]).reshape(256, 1),
        "bg3": f(inputs["bg3"]).reshape(128, 1),
        "bg4c": f(inputs["bg4"]).reshape(E, 1),
        "ones16": np.ones((E, 1), dtype=np.float32),
        "ident": np.eye(128, dtype=np.float32),
    }
    in_maps = []
    for c in range(NCORES):
        m = dict(shared)
        m["xT"] = np.ascontiguousarray(x[c * BC:(c + 1) * BC, :].T)
        in_maps.append(m)
    return in_maps


def kernel(**inputs) -> np.ndarray:
    nc = get_program(has_be1=bool(np.any(np.asarray(inputs["be1"]))),
                     has_be2=bool(np.any(np.asarray(inputs["be2"]))),
                     has_bg4=bool(np.any(np.asarray(inputs["bg4"]))),
                     has_bg123=bool(np.any(np.asarray(inputs["bg1"]))
                                    or np.any(np.asarray(inputs["bg2"]))
                                    or np.any(np.asarray(inputs["bg3"]))))
    in_maps = make_in_maps(inputs)
    res = run_bass_kernel_spmd(nc, in_maps, core_ids=list(range(NCORES)))
    out = np.empty((B, O), dtype=np.float32)
    for c in range(NCORES):
        out[c * BC:(c + 1) * BC, :] = res.results[c]["outT"].T
    return out
